# revision 1
# baseline (speedup 1.0000x reference)
"""GQA per-token attention kernel for Trainium2, 8-core data-parallel.

Reference computation (per token, no cross-token mixing):
  q = x @ Wq.T + bq -> [16 rows of 128]   (rows = (g, kh) flattened)
  k = x @ Wk.T + bk -> [4 heads of 128]
  v = x @ Wv.T + bv -> [4 heads of 128]
  att[r, j] = softmax_j(q_r . k_j / sqrt(128))
  attn_out_r = sum_j att[r, j] * v_j
  y = attn_out @ Wo.T + bo

Sharding: batch*seq = 16384 tokens split contiguously across 8 cores.
Device layout: tokens on SBUF partitions (128/tile); contraction dims on
partitions for matmuls (x pre-transposed on host). Matmuls in bf16 with
fp32 PSUM accumulation; biases folded in as K=1 ones-row matmuls;
per-token attention on DVE/ACT; PE transposes attn_out for the O-proj.
The attention+transpose work for subtile st is emitted after subtile
st+1's matmuls so the PE never stalls waiting on the DVE chain.
"""

import numpy as np
import ml_dtypes

import concourse.bacc as bacc
import concourse.tile as tile
import concourse.mybir as mybir
from concourse.bass_utils import run_bass_kernel_spmd

N_CORES = 8
HID = 2048
D = 128
HC = HID // D            # 16 hidden chunks
QROWS = 16               # q feature chunks (g * kh)
KVH = 4                  # kv heads
TOK_TOTAL = 16384
TOK_CORE = TOK_TOTAL // N_CORES   # 2048
N_MACRO = 2
TOK_MACRO = TOK_CORE // N_MACRO   # 1024
N_ST = TOK_MACRO // 128           # 8 subtiles per macro

BF = mybir.dt.bfloat16
F32 = mybir.dt.float32
AX = mybir.AxisListType
AF = mybir.ActivationFunctionType
INV_SQRT_D = 1.0 / np.sqrt(128.0)

_CACHED = {}


def _build_nc(mm_bufs=6, tr_bufs=2, xt_bufs=3, qkv_bufs=3, av_bufs=4, y_bufs=3,
              qkv_hc=HC, o_ofc=QROWS, pipeline=True):
    nc = bacc.Bacc("TRN2", target_bir_lowering=False, num_devices=N_CORES)

    xt_d = nc.dram_tensor("xt", [HC, D, TOK_CORE], BF, kind="ExternalInput")
    wq_d = nc.dram_tensor("wq", [HC, D, HID], BF, kind="ExternalInput")
    wkv_d = nc.dram_tensor("wkv", [HC, D, 1024], BF, kind="ExternalInput")
    wo_d = nc.dram_tensor("wo", [HC, D, HID], BF, kind="ExternalInput")
    bq_d = nc.dram_tensor("bq", [1, HID], BF, kind="ExternalInput")
    bkv_d = nc.dram_tensor("bkv", [1, 1024], BF, kind="ExternalInput")
    bo_d = nc.dram_tensor("bo", [1, HID], BF, kind="ExternalInput")
    id_d = nc.dram_tensor("ident", [D, D], BF, kind="ExternalInput")
    ones_d = nc.dram_tensor("ones", [1, D], BF, kind="ExternalInput")
    y_d = nc.dram_tensor("y", [TOK_CORE, HID], F32, kind="ExternalOutput")

    with tile.TileContext(nc) as tc:
        with (
            tc.tile_pool(name="const", bufs=1) as constp,
            tc.tile_pool(name="wbig", bufs=1) as wbigp,
            tc.tile_pool(name="wkvp", bufs=1) as wkvp,
            tc.tile_pool(name="xtp", bufs=xt_bufs) as xtp,
            tc.tile_pool(name="qkv", bufs=qkv_bufs) as qkvp,
            tc.tile_pool(name="attnT", bufs=1) as attnp,
            tc.tile_pool(name="av", bufs=av_bufs) as avp,
            tc.tile_pool(name="small", bufs=3) as smallp,
            tc.tile_pool(name="ysb", bufs=y_bufs) as yp,
            tc.tile_pool(name="mm", bufs=mm_bufs, space="PSUM") as mmp,
            tc.tile_pool(name="tr", bufs=tr_bufs, space="PSUM") as trp,
        ):
            ident = constp.tile([D, D], BF, tag="ident")
            nc.sync.dma_start(out=ident[:], in_=id_d[:])
            ones = constp.tile([1, D], BF, tag="ones")
            nc.sync.dma_start(out=ones[:], in_=ones_d[:])
            bq_s = constp.tile([1, HID], BF, tag="bq")
            nc.sync.dma_start(out=bq_s[:], in_=bq_d[:])
            bkv_s = constp.tile([1, 1024], BF, tag="bkv")
            nc.sync.dma_start(out=bkv_s[:], in_=bkv_d[:])
            bo_s = constp.tile([1, HID], BF, tag="bo")
            nc.sync.dma_start(out=bo_s[:], in_=bo_d[:])

            def attn_and_transpose(st, attnT, q_sb, k_sb, v_sb):
                """Per-token attention for one 128-token subtile, then PE
                transposes of attn_out into attnT[:, :, st-slice]."""
                q3 = q_sb[:].rearrange("p (g d) -> p g d", g=QROWS)
                k3 = k_sb[:].rearrange("p (j d) -> p j d", j=KVH)
                v3 = v_sb[:].rearrange("p (j d) -> p j d", j=KVH)

                logits = smallp.tile([128, QROWS, KVH], F32, tag="lg", name="lg")
                for j in range(KVH):
                    prod = avp.tile([128, QROWS, D], BF, tag="av", name=f"pr{j}")
                    nc.vector.tensor_mul(
                        out=prod[:], in0=q3,
                        in1=k3[:, j : j + 1, :].broadcast_to((128, QROWS, D)),
                    )
                    nc.vector.reduce_sum(out=logits[:, :, j], in_=prod[:], axis=AX.X)

                e = smallp.tile([128, QROWS, KVH], F32, tag="e", name="e")
                nc.scalar.activation(out=e[:], in_=logits[:], func=AF.Exp,
                                     scale=float(INV_SQRT_D))
                s = smallp.tile([128, QROWS], F32, tag="s", name="s")
                nc.vector.reduce_sum(out=s[:], in_=e[:], axis=AX.X)
                r = smallp.tile([128, QROWS], F32, tag="r", name="r")
                nc.vector.reciprocal(out=r[:], in_=s[:])
                att = smallp.tile([128, QROWS, KVH], BF, tag="att", name="att")
                nc.vector.tensor_mul(
                    out=att[:], in0=e[:],
                    in1=r[:, :, None].broadcast_to((128, QROWS, KVH)),
                )

                acc = avp.tile([128, QROWS, D], BF, tag="av", name="acc")
                nc.vector.tensor_mul(
                    out=acc[:],
                    in0=v3[:, 0:1, :].broadcast_to((128, QROWS, D)),
                    in1=att[:, :, 0:1].broadcast_to((128, QROWS, D)),
                )
                for j in range(1, KVH):
                    prod = avp.tile([128, QROWS, D], BF, tag="av", name=f"pv{j}")
                    nc.vector.tensor_mul(
                        out=prod[:],
                        in0=v3[:, j : j + 1, :].broadcast_to((128, QROWS, D)),
                        in1=att[:, :, j : j + 1].broadcast_to((128, QROWS, D)),
                    )
                    nc.vector.tensor_add(out=acc[:], in0=acc[:], in1=prod[:])

                for tg in range(4):
                    tr = trp.tile([128, 4, D], BF, tag="tr", name=f"tr{tg}")
                    for i in range(4):
                        ofc = tg * 4 + i
                        nc.tensor.transpose(tr[:, i, :], acc[:, ofc, :], ident[:])
                    nc.scalar.copy(
                        out=attnT[:, tg * 4 : (tg + 1) * 4,
                                  st * 128 : (st + 1) * 128],
                        in_=tr[:],
                    )

            for mac in range(N_MACRO):
                wq = wbigp.tile([D, HC, HID], BF, tag="wbig", name="wq")
                nc.sync.dma_start(out=wq[:], in_=wq_d.rearrange("c p n -> p c n"))
                wkv = wkvp.tile([D, HC, 1024], BF, tag="wkv", name="wkv")
                nc.sync.dma_start(out=wkv[:], in_=wkv_d.rearrange("c p n -> p c n"))
                attnT = attnp.tile([D, QROWS, TOK_MACRO], BF, tag="attnT",
                                   name="attnT")

                pending = None
                for st in range(N_ST):
                    tok0 = mac * TOK_MACRO + st * 128
                    xt = xtp.tile([D, HC, 128], BF, tag="xt", name="xt")
                    nc.sync.dma_start(
                        out=xt[:],
                        in_=xt_d.rearrange("c p t -> p c t")[:, :, tok0 : tok0 + 128],
                    )

                    # ---- QKV projections: out[tok, of] in PSUM ----
                    q_ps = [mmp.tile([128, 512], F32, tag="mm", name=f"qps{og}")
                            for og in range(4)]
                    k_ps = mmp.tile([128, 512], F32, tag="mm", name="kps")
                    v_ps = mmp.tile([128, 512], F32, tag="mm", name="vps")
                    for og in range(4):
                        nc.tensor.matmul(
                            q_ps[og][:], lhsT=ones[:],
                            rhs=bq_s[:, og * 512 : (og + 1) * 512],
                            start=True, stop=False,
                        )
                    nc.tensor.matmul(k_ps[:], lhsT=ones[:], rhs=bkv_s[:, 0:512],
                                     start=True, stop=False)
                    nc.tensor.matmul(v_ps[:], lhsT=ones[:], rhs=bkv_s[:, 512:1024],
                                     start=True, stop=False)
                    for hc in range(qkv_hc):
                        lhs = xt[:, hc, :]
                        last = hc == qkv_hc - 1
                        for og in range(4):
                            nc.tensor.matmul(
                                q_ps[og][:], lhsT=lhs,
                                rhs=wq[:, hc, og * 512 : (og + 1) * 512],
                                start=False, stop=last,
                            )
                        nc.tensor.matmul(k_ps[:], lhsT=lhs, rhs=wkv[:, hc, 0:512],
                                         start=False, stop=last)
                        nc.tensor.matmul(v_ps[:], lhsT=lhs, rhs=wkv[:, hc, 512:1024],
                                         start=False, stop=last)

                    q_sb = qkvp.tile([128, HID], BF, tag="q", name="q_sb")
                    k_sb = qkvp.tile([128, 512], BF, tag="k", name="k_sb")
                    v_sb = qkvp.tile([128, 512], BF, tag="v", name="v_sb")
                    for og in range(4):
                        nc.scalar.copy(out=q_sb[:, og * 512 : (og + 1) * 512],
                                       in_=q_ps[og][:])
                    nc.scalar.copy(out=k_sb[:], in_=k_ps[:])
                    nc.scalar.copy(out=v_sb[:], in_=v_ps[:])

                    # one-subtile software pipeline: emit st-1's attention and
                    # transposes after st's matmuls so PE stays busy while the
                    # DVE works on st-1.
                    if pipeline:
                        if pending is not None:
                            pending()
                        pending = (lambda st=st, q=q_sb, k=k_sb, v=v_sb:
                                   attn_and_transpose(st, attnT, q, k, v))
                    else:
                        attn_and_transpose(st, attnT, q_sb, k_sb, v_sb)
                if pipeline:
                    pending()

                # ---- O projection for this macro ----
                wo = wbigp.tile([D, HC, HID], BF, tag="wbig", name="wo")
                nc.sync.dma_start(out=wo[:], in_=wo_d.rearrange("c p n -> p c n"))
                for st in range(N_ST):
                    tok0 = mac * TOK_MACRO + st * 128
                    y_ps = [mmp.tile([128, 512], F32, tag="mm", name=f"yps{og}")
                            for og in range(4)]
                    for og in range(4):
                        nc.tensor.matmul(
                            y_ps[og][:], lhsT=ones[:],
                            rhs=bo_s[:, og * 512 : (og + 1) * 512],
                            start=True, stop=False,
                        )
                    for ofc in range(o_ofc):
                        lhs = attnT[:, ofc, st * 128 : (st + 1) * 128]
                        last = ofc == o_ofc - 1
                        for og in range(4):
                            nc.tensor.matmul(
                                y_ps[og][:], lhsT=lhs,
                                rhs=wo[:, ofc, og * 512 : (og + 1) * 512],
                                start=False, stop=last,
                            )
                    for og in range(4):
                        y_sb = yp.tile([128, 512], F32, tag="y", name=f"ysb{og}")
                        nc.scalar.copy(out=y_sb[:], in_=y_ps[og][:])
                        nc.sync.dma_start(
                            out=y_d[tok0 : tok0 + 128, og * 512 : (og + 1) * 512],
                            in_=y_sb[:],
                        )

    nc.finalize()
    return nc


def _get_nc():
    if "nc" not in _CACHED:
        _CACHED["nc"] = _build_nc()
    return _CACHED["nc"]


def _prep_inputs(x, Wq, bq, Wk, bk, Wv, bv, Wo, bo):
    bf16 = ml_dtypes.bfloat16
    xf = np.ascontiguousarray(x.reshape(TOK_TOTAL, HID))
    shared = {
        "wq": np.ascontiguousarray(Wq.T.reshape(HC, D, HID)).astype(bf16),
        "wkv": np.ascontiguousarray(
            np.concatenate([Wk.T, Wv.T], axis=1).reshape(HC, D, 1024)
        ).astype(bf16),
        "wo": np.ascontiguousarray(Wo.T.reshape(HC, D, HID)).astype(bf16),
        "bq": bq.reshape(1, HID).astype(bf16),
        "bkv": np.concatenate([bk, bv]).reshape(1, 1024).astype(bf16),
        "bo": bo.reshape(1, HID).astype(bf16),
        "ident": np.eye(D, dtype=np.float32).astype(bf16),
        "ones": np.ones((1, D), dtype=np.float32).astype(bf16),
    }
    in_maps = []
    for c in range(N_CORES):
        xs = xf[c * TOK_CORE : (c + 1) * TOK_CORE]
        xt = np.ascontiguousarray(xs.T.reshape(HC, D, TOK_CORE)).astype(bf16)
        in_maps.append({"xt": xt, **shared})
    return in_maps


def kernel(x, Wq, bq, Wk, bk, Wv, bv, Wo, bo):
    x = np.asarray(x, dtype=np.float32)
    nc = _get_nc()
    in_maps = _prep_inputs(np.asarray(x), np.asarray(Wq), np.asarray(bq),
                           np.asarray(Wk), np.asarray(bk), np.asarray(Wv),
                           np.asarray(bv), np.asarray(Wo), np.asarray(bo))
    res = run_bass_kernel_spmd(nc, in_maps, core_ids=list(range(N_CORES)))
    y = np.concatenate([r["y"] for r in res.results], axis=0)
    return y.reshape(x.shape)



# revision 2
# speedup vs baseline: 22499.9736x; 22499.9736x over previous
"""GQA per-token attention kernel for Trainium2, 8-core data-parallel. v2.

Single-phase design (vs v1's two-macro phases):
  - All weights (Wq, Wkv, Wo) SBUF-resident, loaded ONCE via chunked DMAs
    so matmuls start as soon as chunk 0 lands (v1 reloaded weights per
    macro: 4 exposed DMA stalls ~85us + HAM re-throttle).
  - No bias matmuls on the PE (v1 spent ~33us in K=1 ones-row matmuls);
    biases are pre-broadcast to [128, N] on host and fused into the
    PSUM->SBUF copy as DVE tensor_adds (q/kv) or GPSIMD adds (y).
  - Projections are uniform column-group accumulations: each group is a
    [128, 1024] PSUM tile accumulated over 16 stationary chunks with
    N=1024 bf16 moving operands. 5 groups per subtile (qA qB kv yA yB)
    from a 3-buf PSUM pool (6 banks) + transpose pool = fits 8 banks
    with no PE stalls on copy-out (verified by hand-scheduling).
  - O-projection interleaved per subtile with lag 2 behind QKV, so the
    PE never drains: steady state PE = 3 qkv groups + 16 transposes +
    2 y groups ~= 35us per 128-token subtile.

Per-token attention math (no cross-token mixing): 16 q rows attend over
4 kv heads per token; softmax over 4 logits; contraction dim 128.
"""

import numpy as np
import ml_dtypes

import concourse.bacc as bacc
import concourse.tile as tile
import concourse.mybir as mybir
from concourse.bass_utils import run_bass_kernel_spmd

N_CORES = 8
HID = 2048
D = 128
HC = HID // D            # 16 hidden chunks
QROWS = 16               # q feature chunks (g * kh)
KVH = 4                  # kv heads
TOK_TOTAL = 16384
TOK_CORE = TOK_TOTAL // N_CORES   # 2048
N_ST = TOK_CORE // 128            # 16 subtiles
LAG_O = 2

BF = mybir.dt.bfloat16
F32 = mybir.dt.float32
AX = mybir.AxisListType
AF = mybir.ActivationFunctionType
INV_SQRT_D = 1.0 / np.sqrt(128.0)

_CACHED = {}


def _build_nc(qk_reduce="vector", y_bias="gpsimd", proj_bufs=6, tr_bufs=2,
              bias_bcast="gpsimd"):
    nc = bacc.Bacc("TRN2", target_bir_lowering=False, num_devices=N_CORES)

    xt_d = nc.dram_tensor("xt", [HC, D, TOK_CORE], BF, kind="ExternalInput")
    wq_d = nc.dram_tensor("wq", [HC, D, HID], BF, kind="ExternalInput")
    wkv_d = nc.dram_tensor("wkv", [HC, D, 1024], BF, kind="ExternalInput")
    wo_d = nc.dram_tensor("wo", [HC, D, HID], BF, kind="ExternalInput")
    bias_rows = 1 if bias_bcast == "gpsimd" else D
    bqb_d = nc.dram_tensor("bqb", [bias_rows, HID], BF, kind="ExternalInput")
    bkvb_d = nc.dram_tensor("bkvb", [bias_rows, 1024], BF, kind="ExternalInput")
    bob_d = nc.dram_tensor("bob", [bias_rows, HID], BF, kind="ExternalInput")
    id_d = nc.dram_tensor("ident", [D, D], BF, kind="ExternalInput")
    y_d = nc.dram_tensor("y", [TOK_CORE, HID], BF, kind="ExternalOutput")

    with tile.TileContext(nc) as tc:
        with (
            tc.tile_pool(name="const", bufs=1) as constp,
            tc.tile_pool(name="wq", bufs=1) as wqp,
            tc.tile_pool(name="wkv", bufs=1) as wkvp,
            tc.tile_pool(name="wo", bufs=1) as wop,
            tc.tile_pool(name="xtp", bufs=2) as xtp,
            tc.tile_pool(name="qkv", bufs=1) as qkvp,
            tc.tile_pool(name="prod", bufs=1) as prodp,
            tc.tile_pool(name="acc", bufs=1) as accp,
            tc.tile_pool(name="small", bufs=1) as smallp,
            tc.tile_pool(name="attnT", bufs=LAG_O) as attnTp,
            tc.tile_pool(name="ysb", bufs=2) as yp,
            tc.tile_pool(name="proj", bufs=proj_bufs, space="PSUM") as projp,  # [128,512] f32 = 1 bank each
            tc.tile_pool(name="tr", bufs=tr_bufs, space="PSUM") as trp,
        ):
            xt_r = xt_d.rearrange("c p t -> p c t")
            xts = {}

            def fetch_xt(st, eng=None):
                # st>=2 rides the Activation-engine DMA queue so it isn't
                # stuck behind the 8MB wo transfer on the sync queue.
                xt = xtp.tile([D, HC, 128], BF, tag="xt", name=f"xt{st}")
                tok0 = st * 128
                (eng or nc.scalar).dma_start(
                    out=xt[:], in_=xt_r[:, :, tok0 : tok0 + 128])
                xts[st] = xt

            # Prefetch the first token subtile BEFORE the 20MB of weights
            # hits the DMA queue, so the first matmul only waits on xt(0)
            # + the first wq column block. Weights load as column blocks
            # (matching the column-group consumption order) in a few big
            # DMAs — v2b's 48 per-chunk DMAs serialized on the ~600ns
            # dma_start issue cost and pushed the first MM to 23us.
            fetch_xt(0, nc.sync)

            wq = wqp.tile([D, HC, HID], BF, tag="wq")
            wkv = wkvp.tile([D, HC, 1024], BF, tag="wkv")
            wo = wop.tile([D, HC, HID], BF, tag="wo")
            wq_r = wq_d.rearrange("c p n -> p c n")
            wkv_r = wkv_d.rearrange("c p n -> p c n")
            wo_r = wo_d.rearrange("c p n -> p c n")
            # Consts go on the Activation engine's DMA queue (second HWDGE)
            # so they land early without delaying the critical weight
            # stream on the sync queue. Biases ship as [1, N] and are
            # partition-broadcast on GPSIMD (idle early) — 5KB of DMA
            # instead of 1.25MB competing with the first weight blocks.
            ident = constp.tile([D, D], BF, tag="ident")
            nc.scalar.dma_start(out=ident[:], in_=id_d[:])
            bqb = constp.tile([D, HID], BF, tag="bqb")
            bkvb = constp.tile([D, 1024], BF, tag="bkvb")
            bob = constp.tile([D, HID], BF, tag="bob")
            if bias_bcast == "gpsimd":
                nc.scalar.dma_start(out=bqb[0:1, :], in_=bqb_d[:])
                nc.scalar.dma_start(out=bkvb[0:1, :], in_=bkvb_d[:])
                nc.scalar.dma_start(out=bob[0:1, :], in_=bob_d[:])
                nc.gpsimd.partition_broadcast(bqb[:], bqb[0:1, :])
                nc.gpsimd.partition_broadcast(bkvb[:], bkvb[0:1, :])
                nc.gpsimd.partition_broadcast(bob[:], bob[0:1, :])
            else:
                nc.scalar.dma_start(out=bqb[:], in_=bqb_d[:])
                nc.scalar.dma_start(out=bkvb[:], in_=bkvb_d[:])
                nc.scalar.dma_start(out=bob[:], in_=bob_d[:])

            # Sync queue is serial at ~320GB/s: emission order == arrival
            # order == consumption order. First wq column in fine pieces so
            # the very first MMs only wait ~0.25MB.
            for rq in range(8):
                nc.sync.dma_start(out=wq[:, rq * 2 : (rq + 1) * 2, 0:512],
                                  in_=wq_r[:, rq * 2 : (rq + 1) * 2, 0:512])
            for og in range(1, 4):
                cs = slice(og * 512, (og + 1) * 512)
                nc.sync.dma_start(out=wq[:, :, cs], in_=wq_r[:, :, cs])
            for og in range(2):
                cs = slice(og * 512, (og + 1) * 512)
                nc.sync.dma_start(out=wkv[:, :, cs], in_=wkv_r[:, :, cs])
            fetch_xt(1, nc.sync)
            for og in range(4):
                cs = slice(og * 512, (og + 1) * 512)
                nc.sync.dma_start(out=wo[:, :, cs], in_=wo_r[:, :, cs])

            def emit_qkv_mm(st):
                """6 column-group accumulations (one PSUM bank each):
                q cols 0:512 .. 1536:2048, kv cols 0:512, 512:1024."""
                if st not in xts:
                    fetch_xt(st)
                xt = xts.pop(st)
                groups = []
                specs = [(wq, og * 512) for og in range(4)] + [
                    (wkv, 0), (wkv, 512)]
                for gi, (w, col0) in enumerate(specs):
                    ps = projp.tile([128, 512], F32, tag="proj",
                                    name=f"ps{st}g{gi}")
                    for hc in range(HC):
                        nc.tensor.matmul(
                            ps[:], lhsT=xt[:, hc, :],
                            rhs=w[:, hc, col0 : col0 + 512],
                            start=(hc == 0), stop=(hc == HC - 1),
                        )
                    groups.append(ps)
                return groups

            def emit_qkv_copy(st, groups):
                q_sb = qkvp.tile([128, HID], BF, tag="q", name=f"q{st}")
                kv_sb = qkvp.tile([128, 1024], BF, tag="kv", name=f"kv{st}")
                for og in range(4):
                    nc.vector.tensor_add(
                        out=q_sb[:, og * 512 : (og + 1) * 512],
                        in0=groups[og][:],
                        in1=bqb[:, og * 512 : (og + 1) * 512],
                    )
                nc.vector.tensor_add(out=kv_sb[:, 0:512], in0=groups[4][:],
                                     in1=bkvb[:, 0:512])
                nc.vector.tensor_add(out=kv_sb[:, 512:1024], in0=groups[5][:],
                                     in1=bkvb[:, 512:1024])
                return q_sb, kv_sb

            def emit_attn(st, q_sb, kv_sb):
                """Per-token attention for subtile st; writes attnT(st)."""
                q3 = q_sb[:].rearrange("p (g d) -> p g d", g=QROWS)
                k3 = kv_sb[:, 0:512].rearrange("p (j d) -> p j d", j=KVH)
                v3 = kv_sb[:, 512:1024].rearrange("p (j d) -> p j d", j=KVH)

                # j-major logits: each reduce writes a contiguous [128,16]
                # slice (strided writes cost +60% per op on the DVE).
                logits = smallp.tile([128, KVH, QROWS], F32, tag="lg",
                                     name=f"lg{st}")
                red = nc.vector if qk_reduce == "vector" else nc.gpsimd
                for j in range(KVH):
                    prod = prodp.tile([128, QROWS, D], BF, tag="prod",
                                      name=f"pr{st}_{j}")
                    nc.vector.tensor_mul(
                        out=prod[:], in0=q3,
                        in1=k3[:, j : j + 1, :].broadcast_to((128, QROWS, D)),
                    )
                    red.reduce_sum(out=logits[:, j, :], in_=prod[:], axis=AX.X)

                e = smallp.tile([128, KVH, QROWS], F32, tag="e", name=f"e{st}")
                nc.scalar.activation(out=e[:], in_=logits[:], func=AF.Exp,
                                     scale=float(INV_SQRT_D))
                s = smallp.tile([128, QROWS], F32, tag="s", name=f"s{st}")
                nc.vector.tensor_add(out=s[:], in0=e[:, 0, :], in1=e[:, 1, :])
                nc.vector.tensor_add(out=s[:], in0=s[:], in1=e[:, 2, :])
                nc.vector.tensor_add(out=s[:], in0=s[:], in1=e[:, 3, :])
                r = smallp.tile([128, QROWS], F32, tag="r", name=f"r{st}")
                nc.vector.reciprocal(out=r[:], in_=s[:])
                att = smallp.tile([128, KVH, QROWS], BF, tag="att",
                                  name=f"att{st}")
                nc.vector.tensor_mul(
                    out=att[:], in0=e[:],
                    in1=r[:, None, :].broadcast_to((128, KVH, QROWS)),
                )

                acc = accp.tile([128, QROWS, D], BF, tag="acc", name=f"ac{st}")
                nc.vector.tensor_mul(
                    out=acc[:],
                    in0=v3[:, 0:1, :].broadcast_to((128, QROWS, D)),
                    in1=att[:, 0, :, None].broadcast_to((128, QROWS, D)),
                )
                for j in range(1, KVH):
                    prod = prodp.tile([128, QROWS, D], BF, tag="prod",
                                      name=f"pv{st}_{j}")
                    nc.vector.tensor_mul(
                        out=prod[:],
                        in0=v3[:, j : j + 1, :].broadcast_to((128, QROWS, D)),
                        in1=att[:, j, :, None].broadcast_to((128, QROWS, D)),
                    )
                    nc.vector.tensor_add(out=acc[:], in0=acc[:], in1=prod[:])

                attnT = attnTp.tile([D, QROWS, 128], BF, tag="attnT",
                                    name=f"aT{st}")
                for tg in range(4):
                    tr = trp.tile([128, 4, D], BF, tag="tr", name=f"tr{st}_{tg}")
                    for i in range(4):
                        nc.tensor.transpose(tr[:, i, :], acc[:, tg * 4 + i, :],
                                            ident[:])
                    nc.scalar.copy(out=attnT[:, tg * 4 : (tg + 1) * 4, :],
                                   in_=tr[:])
                return attnT

            def emit_oproj(st, attnT):
                tok0 = st * 128
                for gi in range(4):
                    col0 = gi * 512
                    ps = projp.tile([128, 512], F32, tag="proj",
                                    name=f"yps{st}g{gi}")
                    for ofc in range(QROWS):
                        nc.tensor.matmul(
                            ps[:], lhsT=attnT[:, ofc, :],
                            rhs=wo[:, ofc, col0 : col0 + 512],
                            start=(ofc == 0), stop=(ofc == QROWS - 1),
                        )
                    y_sb = yp.tile([128, 512], BF, tag="y", name=f"y{st}g{gi}")
                    if y_bias == "gpsimd":
                        nc.scalar.copy(out=y_sb[:], in_=ps[:])
                        nc.gpsimd.tensor_add(out=y_sb[:], in0=y_sb[:],
                                             in1=bob[:, col0 : col0 + 512])
                    else:
                        nc.vector.tensor_add(out=y_sb[:], in0=ps[:],
                                             in1=bob[:, col0 : col0 + 512])
                    nc.sync.dma_start(
                        out=y_d[tok0 : tok0 + 128, col0 : col0 + 512],
                        in_=y_sb[:],
                    )

            # Software pipeline, lag 1 for attention, lag 2 for O-proj.
            # Emission order within a step is load-bearing:
            #   attn(s-1) must precede copies(s) on the DVE (qkv bufs=1),
            #   and O(s-2) comes last so its PSUM allocs trail the step's
            #   qkv allocs in the proj ring.
            groups = {}
            qkv_sb = {}
            attnT_t = {}
            for s in range(N_ST + LAG_O):
                if s < N_ST:
                    groups[s] = emit_qkv_mm(s)
                if 1 <= s <= N_ST:
                    q_sb, kv_sb = qkv_sb.pop(s - 1)
                    attnT_t[s - 1] = emit_attn(s - 1, q_sb, kv_sb)
                if s < N_ST:
                    qkv_sb[s] = emit_qkv_copy(s, groups.pop(s))
                if s >= LAG_O:
                    emit_oproj(s - LAG_O, attnT_t.pop(s - LAG_O))

    nc.finalize()
    return nc


def _get_nc():
    if "nc" not in _CACHED:
        _CACHED["nc"] = _build_nc()
    return _CACHED["nc"]


def _prep_inputs(x, Wq, bq, Wk, bk, Wv, bv, Wo, bo):
    bf16 = ml_dtypes.bfloat16
    xf = np.ascontiguousarray(np.asarray(x).reshape(TOK_TOTAL, HID))
    shared = {
        "wq": np.ascontiguousarray(np.asarray(Wq).T.reshape(HC, D, HID)).astype(bf16),
        "wkv": np.ascontiguousarray(
            np.concatenate([np.asarray(Wk).T, np.asarray(Wv).T], axis=1)
            .reshape(HC, D, 1024)
        ).astype(bf16),
        "wo": np.ascontiguousarray(np.asarray(Wo).T.reshape(HC, D, HID)).astype(bf16),
        "bqb": np.asarray(bq).reshape(1, HID).astype(bf16),
        "bkvb": np.concatenate([np.asarray(bk), np.asarray(bv)])
        .reshape(1, 1024).astype(bf16),
        "bob": np.asarray(bo).reshape(1, HID).astype(bf16),
        "ident": np.eye(D, dtype=np.float32).astype(bf16),
    }
    in_maps = []
    for c in range(N_CORES):
        xs = xf[c * TOK_CORE : (c + 1) * TOK_CORE]
        xt = np.ascontiguousarray(xs.T.reshape(HC, D, TOK_CORE)).astype(bf16)
        in_maps.append({"xt": xt, **shared})
    return in_maps


def kernel(x, Wq, bq, Wk, bk, Wv, bv, Wo, bo):
    x = np.asarray(x, dtype=np.float32)
    nc = _get_nc()
    in_maps = _prep_inputs(x, Wq, bq, Wk, bk, Wv, bv, Wo, bo)
    res = run_bass_kernel_spmd(nc, in_maps, core_ids=list(range(N_CORES)))
    y = np.concatenate([r["y"].astype(np.float32) for r in res.results], axis=0)
    return y.reshape(x.shape)


# revision 3
# speedup vs baseline: 22654.3685x; 1.0069x over previous
"""GQA per-token attention kernel for Trainium2, 8-core data-parallel. v2.

Single-phase design (vs v1's two-macro phases):
  - All weights (Wq, Wkv, Wo) SBUF-resident, loaded ONCE via chunked DMAs
    so matmuls start as soon as chunk 0 lands (v1 reloaded weights per
    macro: 4 exposed DMA stalls ~85us + HAM re-throttle).
  - No bias matmuls on the PE (v1 spent ~33us in K=1 ones-row matmuls);
    biases are pre-broadcast to [128, N] on host and fused into the
    PSUM->SBUF copy as DVE tensor_adds (q/kv) or GPSIMD adds (y).
  - Projections are uniform column-group accumulations: each group is a
    [128, 512] PSUM tile (one bank — the matmul-output limit) accumulated
    over 16 stationary chunks with N=512 bf16 moving operands. 10 groups
    per subtile (4 q, 2 kv, 4 y) ride a 6-buf PSUM ring + transpose pool
    = 8 banks, with ring reuse distances chosen so no matmul waits on a
    copy-out.
  - Two DMA queues: big weight stream on the sync queue in consumption
    order; ident/biases/steady-state x tiles on the Activation-engine
    queue. Biases ship as [1, N] rows and are partition-broadcast on
    GPSIMD, which also applies the output bias (third elementwise engine
    keeps the DVE under the PE's per-subtile budget).
  - O-projection interleaved per subtile with lag 2 behind QKV, so the
    PE never drains: steady state PE = 3 qkv groups + 16 transposes +
    2 y groups ~= 35us per 128-token subtile.

Per-token attention math (no cross-token mixing): 16 q rows attend over
4 kv heads per token; softmax over 4 logits; contraction dim 128.
"""

import numpy as np
import ml_dtypes

import concourse.bacc as bacc
import concourse.tile as tile
import concourse.mybir as mybir
from concourse.bass_utils import run_bass_kernel_spmd

N_CORES = 8
HID = 2048
D = 128
HC = HID // D            # 16 hidden chunks
QROWS = 16               # q feature chunks (g * kh)
KVH = 4                  # kv heads
TOK_TOTAL = 16384
TOK_CORE = TOK_TOTAL // N_CORES   # 2048
N_ST = TOK_CORE // 128            # 16 subtiles
LAG_O = 2

BF = mybir.dt.bfloat16
F32 = mybir.dt.float32
AX = mybir.AxisListType
AF = mybir.ActivationFunctionType
INV_SQRT_D = 1.0 / np.sqrt(128.0)

_CACHED = {}


def _build_nc(qk_reduce="vector", y_bias="gpsimd", proj_bufs=6, tr_bufs=2,
              bias_bcast="gpsimd"):
    nc = bacc.Bacc("TRN2", target_bir_lowering=False, num_devices=N_CORES)

    xt_d = nc.dram_tensor("xt", [HC, D, TOK_CORE], BF, kind="ExternalInput")
    wq_d = nc.dram_tensor("wq", [HC, D, HID], BF, kind="ExternalInput")
    wkv_d = nc.dram_tensor("wkv", [HC, D, 1024], BF, kind="ExternalInput")
    wo_d = nc.dram_tensor("wo", [HC, D, HID], BF, kind="ExternalInput")
    bias_rows = 1 if bias_bcast == "gpsimd" else D
    bqb_d = nc.dram_tensor("bqb", [bias_rows, HID], BF, kind="ExternalInput")
    bkvb_d = nc.dram_tensor("bkvb", [bias_rows, 1024], BF, kind="ExternalInput")
    bob_d = nc.dram_tensor("bob", [bias_rows, HID], BF, kind="ExternalInput")
    id_d = nc.dram_tensor("ident", [D, D], BF, kind="ExternalInput")
    y_d = nc.dram_tensor("y", [TOK_CORE, HID], BF, kind="ExternalOutput")

    with tile.TileContext(nc) as tc:
        with (
            tc.tile_pool(name="const", bufs=1) as constp,
            tc.tile_pool(name="wq", bufs=1) as wqp,
            tc.tile_pool(name="wkv", bufs=1) as wkvp,
            tc.tile_pool(name="wo", bufs=1) as wop,
            tc.tile_pool(name="xtp", bufs=2) as xtp,
            tc.tile_pool(name="qkv", bufs=1) as qkvp,
            tc.tile_pool(name="prod", bufs=1) as prodp,
            tc.tile_pool(name="acc", bufs=1) as accp,
            tc.tile_pool(name="small", bufs=1) as smallp,
            tc.tile_pool(name="attnT", bufs=LAG_O) as attnTp,
            tc.tile_pool(name="ysb", bufs=2) as yp,
            tc.tile_pool(name="proj", bufs=proj_bufs, space="PSUM") as projp,  # [128,512] f32 = 1 bank each
            tc.tile_pool(name="tr", bufs=tr_bufs, space="PSUM") as trp,
        ):
            xt_r = xt_d.rearrange("c p t -> p c t")
            xts = {}

            def fetch_xt(st, eng=None):
                # st>=2 rides the Activation-engine DMA queue so it isn't
                # stuck behind the 8MB wo transfer on the sync queue.
                xt = xtp.tile([D, HC, 128], BF, tag="xt", name=f"xt{st}")
                tok0 = st * 128
                (eng or nc.scalar).dma_start(
                    out=xt[:], in_=xt_r[:, :, tok0 : tok0 + 128])
                xts[st] = xt

            # Prefetch the first token subtile BEFORE the 20MB of weights
            # hits the DMA queue, so the first matmul only waits on xt(0)
            # + the first wq column block. Weights load as column blocks
            # (matching the column-group consumption order) in a few big
            # DMAs — v2b's 48 per-chunk DMAs serialized on the ~600ns
            # dma_start issue cost and pushed the first MM to 23us.
            fetch_xt(0, nc.sync)

            wq = wqp.tile([D, HC, HID], BF, tag="wq")
            wkv = wkvp.tile([D, HC, 1024], BF, tag="wkv")
            wo = wop.tile([D, HC, HID], BF, tag="wo")
            wq_r = wq_d.rearrange("c p n -> p c n")
            wkv_r = wkv_d.rearrange("c p n -> p c n")
            wo_r = wo_d.rearrange("c p n -> p c n")
            # Consts go on the Activation engine's DMA queue (second HWDGE)
            # so they land early without delaying the critical weight
            # stream on the sync queue. Biases ship as [1, N] and are
            # partition-broadcast on GPSIMD (idle early) — 5KB of DMA
            # instead of 1.25MB competing with the first weight blocks.
            ident = constp.tile([D, D], BF, tag="ident")
            nc.scalar.dma_start(out=ident[:], in_=id_d[:])
            bqb = constp.tile([D, HID], BF, tag="bqb")
            bkvb = constp.tile([D, 1024], BF, tag="bkvb")
            bob = constp.tile([D, HID], BF, tag="bob")
            if bias_bcast == "gpsimd":
                nc.scalar.dma_start(out=bqb[0:1, :], in_=bqb_d[:])
                nc.scalar.dma_start(out=bkvb[0:1, :], in_=bkvb_d[:])
                nc.scalar.dma_start(out=bob[0:1, :], in_=bob_d[:])
                nc.gpsimd.partition_broadcast(bqb[:], bqb[0:1, :])
                nc.gpsimd.partition_broadcast(bkvb[:], bkvb[0:1, :])
                nc.gpsimd.partition_broadcast(bob[:], bob[0:1, :])
            else:
                nc.scalar.dma_start(out=bqb[:], in_=bqb_d[:])
                nc.scalar.dma_start(out=bkvb[:], in_=bkvb_d[:])
                nc.scalar.dma_start(out=bob[:], in_=bob_d[:])

            # Sync queue is serial at ~320GB/s: emission order == arrival
            # order == consumption order. First wq column in fine pieces so
            # the very first MMs only wait ~0.25MB.
            for rq in range(8):
                nc.sync.dma_start(out=wq[:, rq * 2 : (rq + 1) * 2, 0:512],
                                  in_=wq_r[:, rq * 2 : (rq + 1) * 2, 0:512])
            for og in range(1, 4):
                cs = slice(og * 512, (og + 1) * 512)
                nc.sync.dma_start(out=wq[:, :, cs], in_=wq_r[:, :, cs])
            for og in range(2):
                cs = slice(og * 512, (og + 1) * 512)
                nc.sync.dma_start(out=wkv[:, :, cs], in_=wkv_r[:, :, cs])
            fetch_xt(1, nc.sync)
            for og in range(4):
                cs = slice(og * 512, (og + 1) * 512)
                nc.sync.dma_start(out=wo[:, :, cs], in_=wo_r[:, :, cs])

            def emit_qkv_mm(st):
                """6 column-group accumulations (one PSUM bank each):
                q cols 0:512 .. 1536:2048, kv cols 0:512, 512:1024."""
                if st not in xts:
                    fetch_xt(st)
                xt = xts.pop(st)
                groups = []
                specs = [(wq, og * 512) for og in range(4)] + [
                    (wkv, 0), (wkv, 512)]
                for gi, (w, col0) in enumerate(specs):
                    ps = projp.tile([128, 512], F32, tag="proj",
                                    name=f"ps{st}g{gi}")
                    for hc in range(HC):
                        nc.tensor.matmul(
                            ps[:], lhsT=xt[:, hc, :],
                            rhs=w[:, hc, col0 : col0 + 512],
                            start=(hc == 0), stop=(hc == HC - 1),
                        )
                    groups.append(ps)
                return groups

            def emit_qkv_copy(st, groups):
                q_sb = qkvp.tile([128, HID], BF, tag="q", name=f"q{st}")
                kv_sb = qkvp.tile([128, 1024], BF, tag="kv", name=f"kv{st}")
                for og in range(4):
                    nc.vector.tensor_add(
                        out=q_sb[:, og * 512 : (og + 1) * 512],
                        in0=groups[og][:],
                        in1=bqb[:, og * 512 : (og + 1) * 512],
                    )
                nc.vector.tensor_add(out=kv_sb[:, 0:512], in0=groups[4][:],
                                     in1=bkvb[:, 0:512])
                nc.vector.tensor_add(out=kv_sb[:, 512:1024], in0=groups[5][:],
                                     in1=bkvb[:, 512:1024])
                return q_sb, kv_sb

            def emit_attn(st, q_sb, kv_sb):
                """Per-token attention for subtile st; writes attnT(st)."""
                q3 = q_sb[:].rearrange("p (g d) -> p g d", g=QROWS)
                k3 = kv_sb[:, 0:512].rearrange("p (j d) -> p j d", j=KVH)
                v3 = kv_sb[:, 512:1024].rearrange("p (j d) -> p j d", j=KVH)

                # j-major logits: each reduce writes a contiguous [128,16]
                # slice (strided writes cost +60% per op on the DVE).
                logits = smallp.tile([128, KVH, QROWS], F32, tag="lg",
                                     name=f"lg{st}")
                red = nc.vector if qk_reduce == "vector" else nc.gpsimd
                for j in range(KVH):
                    prod = prodp.tile([128, QROWS, D], BF, tag="prod",
                                      name=f"pr{st}_{j}")
                    nc.vector.tensor_mul(
                        out=prod[:], in0=q3,
                        in1=k3[:, j : j + 1, :].broadcast_to((128, QROWS, D)),
                    )
                    red.reduce_sum(out=logits[:, j, :], in_=prod[:], axis=AX.X)

                e = smallp.tile([128, KVH, QROWS], F32, tag="e", name=f"e{st}")
                nc.scalar.activation(out=e[:], in_=logits[:], func=AF.Exp,
                                     scale=float(INV_SQRT_D))
                s = smallp.tile([128, QROWS], F32, tag="s", name=f"s{st}")
                nc.vector.tensor_add(out=s[:], in0=e[:, 0, :], in1=e[:, 1, :])
                nc.vector.tensor_add(out=s[:], in0=s[:], in1=e[:, 2, :])
                nc.vector.tensor_add(out=s[:], in0=s[:], in1=e[:, 3, :])
                r = smallp.tile([128, QROWS], F32, tag="r", name=f"r{st}")
                nc.vector.reciprocal(out=r[:], in_=s[:])
                att = smallp.tile([128, KVH, QROWS], BF, tag="att",
                                  name=f"att{st}")
                nc.vector.tensor_mul(
                    out=att[:], in0=e[:],
                    in1=r[:, None, :].broadcast_to((128, KVH, QROWS)),
                )

                acc = accp.tile([128, QROWS, D], BF, tag="acc", name=f"ac{st}")
                nc.vector.tensor_mul(
                    out=acc[:],
                    in0=v3[:, 0:1, :].broadcast_to((128, QROWS, D)),
                    in1=att[:, 0, :, None].broadcast_to((128, QROWS, D)),
                )
                for j in range(1, KVH):
                    prod = prodp.tile([128, QROWS, D], BF, tag="prod",
                                      name=f"pv{st}_{j}")
                    nc.vector.tensor_mul(
                        out=prod[:],
                        in0=v3[:, j : j + 1, :].broadcast_to((128, QROWS, D)),
                        in1=att[:, j, :, None].broadcast_to((128, QROWS, D)),
                    )
                    nc.vector.tensor_add(out=acc[:], in0=acc[:], in1=prod[:])

                attnT = attnTp.tile([D, QROWS, 128], BF, tag="attnT",
                                    name=f"aT{st}")
                for tg in range(4):
                    tr = trp.tile([128, 4, D], BF, tag="tr", name=f"tr{st}_{tg}")
                    for i in range(4):
                        nc.tensor.transpose(tr[:, i, :], acc[:, tg * 4 + i, :],
                                            ident[:])
                    nc.scalar.copy(out=attnT[:, tg * 4 : (tg + 1) * 4, :],
                                   in_=tr[:])
                return attnT

            def emit_oproj(st, attnT):
                tok0 = st * 128
                for gi in range(4):
                    col0 = gi * 512
                    ps = projp.tile([128, 512], F32, tag="proj",
                                    name=f"yps{st}g{gi}")
                    for ofc in range(QROWS):
                        nc.tensor.matmul(
                            ps[:], lhsT=attnT[:, ofc, :],
                            rhs=wo[:, ofc, col0 : col0 + 512],
                            start=(ofc == 0), stop=(ofc == QROWS - 1),
                        )
                    y_sb = yp.tile([128, 512], BF, tag="y", name=f"y{st}g{gi}")
                    if y_bias == "gpsimd":
                        nc.scalar.copy(out=y_sb[:], in_=ps[:])
                        nc.gpsimd.tensor_add(out=y_sb[:], in0=y_sb[:],
                                             in1=bob[:, col0 : col0 + 512])
                    else:
                        nc.vector.tensor_add(out=y_sb[:], in0=ps[:],
                                             in1=bob[:, col0 : col0 + 512])
                    nc.sync.dma_start(
                        out=y_d[tok0 : tok0 + 128, col0 : col0 + 512],
                        in_=y_sb[:],
                    )

            # Software pipeline, lag 1 for attention, lag 2 for O-proj.
            # Emission order within a step is load-bearing:
            #   attn(s-1) must precede copies(s) on the DVE (qkv bufs=1),
            #   and O(s-2) comes last so its PSUM allocs trail the step's
            #   qkv allocs in the proj ring.
            groups = {}
            qkv_sb = {}
            attnT_t = {}
            for s in range(N_ST + LAG_O):
                if s < N_ST:
                    groups[s] = emit_qkv_mm(s)
                if 1 <= s <= N_ST:
                    q_sb, kv_sb = qkv_sb.pop(s - 1)
                    attnT_t[s - 1] = emit_attn(s - 1, q_sb, kv_sb)
                if s < N_ST:
                    qkv_sb[s] = emit_qkv_copy(s, groups.pop(s))
                if s >= LAG_O:
                    emit_oproj(s - LAG_O, attnT_t.pop(s - LAG_O))

    nc.finalize()
    return nc


def _get_nc():
    if "nc" not in _CACHED:
        _CACHED["nc"] = _build_nc()
    return _CACHED["nc"]


def _prep_inputs(x, Wq, bq, Wk, bk, Wv, bv, Wo, bo):
    bf16 = ml_dtypes.bfloat16
    xf = np.ascontiguousarray(np.asarray(x).reshape(TOK_TOTAL, HID))
    shared = {
        "wq": np.ascontiguousarray(np.asarray(Wq).T.reshape(HC, D, HID)).astype(bf16),
        "wkv": np.ascontiguousarray(
            np.concatenate([np.asarray(Wk).T, np.asarray(Wv).T], axis=1)
            .reshape(HC, D, 1024)
        ).astype(bf16),
        "wo": np.ascontiguousarray(np.asarray(Wo).T.reshape(HC, D, HID)).astype(bf16),
        "bqb": np.asarray(bq).reshape(1, HID).astype(bf16),
        "bkvb": np.concatenate([np.asarray(bk), np.asarray(bv)])
        .reshape(1, 1024).astype(bf16),
        "bob": np.asarray(bo).reshape(1, HID).astype(bf16),
        "ident": np.eye(D, dtype=np.float32).astype(bf16),
    }
    in_maps = []
    for c in range(N_CORES):
        xs = xf[c * TOK_CORE : (c + 1) * TOK_CORE]
        xt = np.ascontiguousarray(xs.T.reshape(HC, D, TOK_CORE)).astype(bf16)
        in_maps.append({"xt": xt, **shared})
    return in_maps


def kernel(x, Wq, bq, Wk, bk, Wv, bv, Wo, bo):
    x = np.asarray(x, dtype=np.float32)
    nc = _get_nc()
    in_maps = _prep_inputs(x, Wq, bq, Wk, bk, Wv, bv, Wo, bo)
    res = run_bass_kernel_spmd(nc, in_maps, core_ids=list(range(N_CORES)))
    y = np.concatenate([r["y"].astype(np.float32) for r in res.results], axis=0)
    return y.reshape(x.shape)


# revision 6
# speedup vs baseline: 22843.9784x; 1.0084x over previous
"""GQA per-token attention kernel for Trainium2, 8-core data-parallel. v2.

Single-phase design (vs v1's two-macro phases):
  - All weights (Wq, Wkv, Wo) SBUF-resident, loaded ONCE via chunked DMAs
    so matmuls start as soon as chunk 0 lands (v1 reloaded weights per
    macro: 4 exposed DMA stalls ~85us + HAM re-throttle).
  - No bias matmuls on the PE (v1 spent ~33us in K=1 ones-row matmuls);
    biases are pre-broadcast to [128, N] on host and fused into the
    PSUM->SBUF copy as DVE tensor_adds (q/kv) or GPSIMD adds (y).
  - Projections are uniform column-group accumulations: each group is a
    [128, 512] PSUM tile (one bank — the matmul-output limit) accumulated
    over 16 stationary chunks with N=512 bf16 moving operands. 10 groups
    per subtile (4 q, 2 kv, 4 y) ride a 6-buf PSUM ring + transpose pool
    = 8 banks, with ring reuse distances chosen so no matmul waits on a
    copy-out. Two DMA queues (sync + Activation-engine) split the weight
    stream from consts/x-tiles; biases ship as [1, N] rows and are
    partition-broadcast on GPSIMD.
  - O-projection interleaved per subtile with lag 2 behind QKV, so the
    PE never drains: steady state PE = 3 qkv groups + 16 transposes +
    2 y groups ~= 35us per 128-token subtile.

Per-token attention math (no cross-token mixing): 16 q rows attend over
4 kv heads per token; softmax over 4 logits; contraction dim 128.
"""

import numpy as np
import ml_dtypes

import concourse.bacc as bacc
import concourse.tile as tile
import concourse.mybir as mybir
from concourse.bass_utils import run_bass_kernel_spmd

N_CORES = 8
HID = 2048
D = 128
HC = HID // D            # 16 hidden chunks
QROWS = 16               # q feature chunks (g * kh)
KVH = 4                  # kv heads
TOK_TOTAL = 16384
TOK_CORE = TOK_TOTAL // N_CORES   # 2048
N_ST = TOK_CORE // 128            # 16 subtiles
LAG_O = 2

BF = mybir.dt.bfloat16
F32 = mybir.dt.float32
AX = mybir.AxisListType
AF = mybir.ActivationFunctionType
INV_SQRT_D = 1.0 / np.sqrt(128.0)

_CACHED = {}


def _build_nc(qk_reduce="vector", y_bias="gpsimd", proj_bufs=6, tr_bufs=2,
              bias_bcast="gpsimd"):
    nc = bacc.Bacc("TRN2", target_bir_lowering=False, num_devices=N_CORES)

    # subtile-major so each [D, HC, 128] slice is 4KB-contiguous per
    # partition (the [HC, D, TOK] layout gave 256B DMA runs ~ 90GB/s)
    xt_d = nc.dram_tensor("xt", [D, N_ST, HC, 128], BF, kind="ExternalInput")
    wq_d = nc.dram_tensor("wq", [HC, D, HID], BF, kind="ExternalInput")
    wkv_d = nc.dram_tensor("wkv", [HC, D, 1024], BF, kind="ExternalInput")
    wo_d = nc.dram_tensor("wo", [HC, D, HID], BF, kind="ExternalInput")
    bias_rows = 1 if bias_bcast == "gpsimd" else D
    bqb_d = nc.dram_tensor("bqb", [bias_rows, HID], BF, kind="ExternalInput")
    bkvb_d = nc.dram_tensor("bkvb", [bias_rows, 1024], BF, kind="ExternalInput")
    bob_d = nc.dram_tensor("bob", [bias_rows, HID], BF, kind="ExternalInput")
    id_d = nc.dram_tensor("ident", [D, D], BF, kind="ExternalInput")
    y_d = nc.dram_tensor("y", [TOK_CORE, HID], BF, kind="ExternalOutput")

    with tile.TileContext(nc) as tc:
        with (
            tc.tile_pool(name="const", bufs=1) as constp,
            tc.tile_pool(name="wq", bufs=1) as wqp,
            tc.tile_pool(name="wkv", bufs=1) as wkvp,
            tc.tile_pool(name="wo", bufs=1) as wop,
            tc.tile_pool(name="xtp", bufs=2) as xtp,
            tc.tile_pool(name="qkv", bufs=1) as qkvp,
            tc.tile_pool(name="prod", bufs=1) as prodp,
            tc.tile_pool(name="acc", bufs=1) as accp,
            tc.tile_pool(name="small", bufs=1) as smallp,
            tc.tile_pool(name="attnT", bufs=LAG_O) as attnTp,
            tc.tile_pool(name="ysb", bufs=4) as yp,
            tc.tile_pool(name="proj", bufs=proj_bufs, space="PSUM") as projp,  # [128,512] f32 = 1 bank each
            tc.tile_pool(name="tr", bufs=tr_bufs, space="PSUM") as trp,
        ):
            xts = {}

            def fetch_xt(st, eng=None):
                # st>=2 rides the Activation-engine DMA queue so it isn't
                # stuck behind the 8MB wo transfer on the sync queue.
                xt = xtp.tile([D, HC, 128], BF, tag="xt", name=f"xt{st}")
                (eng or nc.scalar).dma_start(
                    out=xt[:], in_=xt_d[:, st, :, :])
                xts[st] = xt

            # Prefetch the first token subtile BEFORE the 20MB of weights
            # hits the DMA queue, so the first matmul only waits on xt(0)
            # + the first wq column block. Weights load as column blocks
            # (matching the column-group consumption order) in a few big
            # DMAs — v2b's 48 per-chunk DMAs serialized on the ~600ns
            # dma_start issue cost and pushed the first MM to 23us.
            fetch_xt(0, nc.sync)

            wq = wqp.tile([D, HC, HID], BF, tag="wq")
            wkv = wkvp.tile([D, HC, 1024], BF, tag="wkv")
            wo = wop.tile([D, HC, HID], BF, tag="wo")
            wq_r = wq_d.rearrange("c p n -> p c n")
            wkv_r = wkv_d.rearrange("c p n -> p c n")
            wo_r = wo_d.rearrange("c p n -> p c n")
            # Consts go on the Activation engine's DMA queue (second HWDGE)
            # so they land early without delaying the critical weight
            # stream on the sync queue. Biases ship as [1, N] and are
            # partition-broadcast on GPSIMD (idle early) — 5KB of DMA
            # instead of 1.25MB competing with the first weight blocks.
            ident = constp.tile([D, D], BF, tag="ident")
            nc.scalar.dma_start(out=ident[:], in_=id_d[:])
            bqb = constp.tile([D, HID], BF, tag="bqb")
            bkvb = constp.tile([D, 1024], BF, tag="bkvb")
            bob = constp.tile([D, HID], BF, tag="bob")
            if bias_bcast == "gpsimd":
                nc.scalar.dma_start(out=bqb[0:1, :], in_=bqb_d[:])
                nc.scalar.dma_start(out=bkvb[0:1, :], in_=bkvb_d[:])
                nc.scalar.dma_start(out=bob[0:1, :], in_=bob_d[:])
                nc.gpsimd.partition_broadcast(bqb[:], bqb[0:1, :])
                nc.gpsimd.partition_broadcast(bkvb[:], bkvb[0:1, :])
                nc.gpsimd.partition_broadcast(bob[:], bob[0:1, :])
            else:
                nc.scalar.dma_start(out=bqb[:], in_=bqb_d[:])
                nc.scalar.dma_start(out=bkvb[:], in_=bkvb_d[:])
                nc.scalar.dma_start(out=bob[:], in_=bob_d[:])

            # Sync queue is serial at ~320GB/s: emission order == arrival
            # order == consumption order. First wq column in fine pieces so
            # the very first MMs only wait ~0.25MB.
            for rq in range(8):
                nc.sync.dma_start(out=wq[:, rq * 2 : (rq + 1) * 2, 0:512],
                                  in_=wq_r[:, rq * 2 : (rq + 1) * 2, 0:512])
            # 1MB half-column pieces: PE waits per piece stay under the
            # 3.4us HAM window, so the clock gate never re-throttles
            # during the DMA-paced first two subtiles.
            for og in range(1, 4):
                for h in range(2):
                    cs = slice(og * 512, (og + 1) * 512)
                    hs = slice(h * 8, (h + 1) * 8)
                    nc.sync.dma_start(out=wq[:, hs, cs], in_=wq_r[:, hs, cs])
            for og in range(2):
                for h in range(2):
                    cs = slice(og * 512, (og + 1) * 512)
                    hs = slice(h * 8, (h + 1) * 8)
                    nc.sync.dma_start(out=wkv[:, hs, cs],
                                      in_=wkv_r[:, hs, cs])
            fetch_xt(1, nc.sync)
            for og in range(4):
                cs = slice(og * 512, (og + 1) * 512)
                nc.sync.dma_start(out=wo[:, :, cs], in_=wo_r[:, :, cs])

            def emit_qkv_mm(st):
                """6 column-group accumulations (one PSUM bank each):
                q cols 0:512 .. 1536:2048, kv cols 0:512, 512:1024."""
                if st not in xts:
                    fetch_xt(st)
                xt = xts.pop(st)
                groups = []
                specs = [(wq, og * 512) for og in range(4)] + [
                    (wkv, 0), (wkv, 512)]
                for gi, (w, col0) in enumerate(specs):
                    ps = projp.tile([128, 512], F32, tag="proj",
                                    name=f"ps{st}g{gi}")
                    for hc in range(HC):
                        nc.tensor.matmul(
                            ps[:], lhsT=xt[:, hc, :],
                            rhs=w[:, hc, col0 : col0 + 512],
                            start=(hc == 0), stop=(hc == HC - 1),
                        )
                    groups.append(ps)
                return groups

            def emit_qkv_copy(st, groups):
                q_sb = qkvp.tile([128, HID], BF, tag="q", name=f"q{st}")
                kv_sb = qkvp.tile([128, 1024], BF, tag="kv", name=f"kv{st}")
                for og in range(4):
                    nc.vector.tensor_add(
                        out=q_sb[:, og * 512 : (og + 1) * 512],
                        in0=groups[og][:],
                        in1=bqb[:, og * 512 : (og + 1) * 512],
                    )
                nc.vector.tensor_add(out=kv_sb[:, 0:512], in0=groups[4][:],
                                     in1=bkvb[:, 0:512])
                nc.vector.tensor_add(out=kv_sb[:, 512:1024], in0=groups[5][:],
                                     in1=bkvb[:, 512:1024])
                return q_sb, kv_sb

            def emit_attn(st, q_sb, kv_sb):
                """Per-token attention for subtile st; writes attnT(st)."""
                q3 = q_sb[:].rearrange("p (g d) -> p g d", g=QROWS)
                k3 = kv_sb[:, 0:512].rearrange("p (j d) -> p j d", j=KVH)
                v3 = kv_sb[:, 512:1024].rearrange("p (j d) -> p j d", j=KVH)

                # j-major logits: each reduce writes a contiguous [128,16]
                # slice (strided writes cost +60% per op on the DVE).
                logits = smallp.tile([128, KVH, QROWS], F32, tag="lg",
                                     name=f"lg{st}")
                red = nc.vector if qk_reduce == "vector" else nc.gpsimd
                for j in range(KVH):
                    prod = prodp.tile([128, QROWS, D], BF, tag="prod",
                                      name=f"pr{st}_{j}")
                    nc.vector.tensor_mul(
                        out=prod[:], in0=q3,
                        in1=k3[:, j : j + 1, :].broadcast_to((128, QROWS, D)),
                    )
                    red.reduce_sum(out=logits[:, j, :], in_=prod[:], axis=AX.X)

                e = smallp.tile([128, KVH, QROWS], F32, tag="e", name=f"e{st}")
                nc.scalar.activation(out=e[:], in_=logits[:], func=AF.Exp,
                                     scale=float(INV_SQRT_D))
                s = smallp.tile([128, QROWS], F32, tag="s", name=f"s{st}")
                nc.vector.tensor_add(out=s[:], in0=e[:, 0, :], in1=e[:, 1, :])
                nc.vector.tensor_add(out=s[:], in0=s[:], in1=e[:, 2, :])
                nc.vector.tensor_add(out=s[:], in0=s[:], in1=e[:, 3, :])
                r = smallp.tile([128, QROWS], F32, tag="r", name=f"r{st}")
                nc.vector.reciprocal(out=r[:], in_=s[:])
                att = smallp.tile([128, KVH, QROWS], BF, tag="att",
                                  name=f"att{st}")
                nc.vector.tensor_mul(
                    out=att[:], in0=e[:],
                    in1=r[:, None, :].broadcast_to((128, KVH, QROWS)),
                )

                acc = accp.tile([128, QROWS, D], BF, tag="acc", name=f"ac{st}")
                nc.vector.tensor_mul(
                    out=acc[:],
                    in0=v3[:, 0:1, :].broadcast_to((128, QROWS, D)),
                    in1=att[:, 0, :, None].broadcast_to((128, QROWS, D)),
                )
                for j in range(1, KVH):
                    prod = prodp.tile([128, QROWS, D], BF, tag="prod",
                                      name=f"pv{st}_{j}")
                    nc.vector.tensor_mul(
                        out=prod[:],
                        in0=v3[:, j : j + 1, :].broadcast_to((128, QROWS, D)),
                        in1=att[:, j, :, None].broadcast_to((128, QROWS, D)),
                    )
                    nc.vector.tensor_add(out=acc[:], in0=acc[:], in1=prod[:])

                attnT = attnTp.tile([D, QROWS, 128], BF, tag="attnT",
                                    name=f"aT{st}")
                for tg in range(4):
                    tr = trp.tile([128, 4, D], BF, tag="tr", name=f"tr{st}_{tg}")
                    for i in range(4):
                        nc.tensor.transpose(tr[:, i, :], acc[:, tg * 4 + i, :],
                                            ident[:])
                    nc.scalar.copy(out=attnT[:, tg * 4 : (tg + 1) * 4, :],
                                   in_=tr[:])
                return attnT

            def emit_oproj(st, attnT):
                tok0 = st * 128
                for gi in range(4):
                    col0 = gi * 512
                    ps = projp.tile([128, 512], F32, tag="proj",
                                    name=f"yps{st}g{gi}")
                    for ofc in range(QROWS):
                        nc.tensor.matmul(
                            ps[:], lhsT=attnT[:, ofc, :],
                            rhs=wo[:, ofc, col0 : col0 + 512],
                            start=(ofc == 0), stop=(ofc == QROWS - 1),
                        )
                    y_sb = yp.tile([128, 512], BF, tag="y", name=f"y{st}g{gi}")
                    if y_bias == "gpsimd" and st < N_ST - 2:
                        nc.scalar.copy(out=y_sb[:], in_=ps[:])
                        nc.gpsimd.tensor_add(out=y_sb[:], in0=y_sb[:],
                                             in1=bob[:, col0 : col0 + 512])
                    else:
                        # drain steps: DVE is idle — one fused add shortens
                        # the last-subtile copy->add->DMA tail chain
                        nc.vector.tensor_add(out=y_sb[:], in0=ps[:],
                                             in1=bob[:, col0 : col0 + 512])
                    nc.sync.dma_start(
                        out=y_d[tok0 : tok0 + 128, col0 : col0 + 512],
                        in_=y_sb[:],
                    )

            # Software pipeline, lag 1 for attention, lag 2 for O-proj.
            # Emission order within a step is load-bearing:
            #   attn(s-1) must precede copies(s) on the DVE (qkv bufs=1),
            #   and O(s-2) comes last so its PSUM allocs trail the step's
            #   qkv allocs in the proj ring.
            groups = {}
            qkv_sb = {}
            attnT_t = {}
            for s in range(N_ST + LAG_O):
                if s < N_ST:
                    groups[s] = emit_qkv_mm(s)
                if 1 <= s <= N_ST:
                    q_sb, kv_sb = qkv_sb.pop(s - 1)
                    attnT_t[s - 1] = emit_attn(s - 1, q_sb, kv_sb)
                if s < N_ST:
                    qkv_sb[s] = emit_qkv_copy(s, groups.pop(s))
                if s >= LAG_O:
                    emit_oproj(s - LAG_O, attnT_t.pop(s - LAG_O))

    nc.finalize()
    return nc


def _get_nc():
    if "nc" not in _CACHED:
        _CACHED["nc"] = _build_nc()
    return _CACHED["nc"]


def _prep_inputs(x, Wq, bq, Wk, bk, Wv, bv, Wo, bo):
    bf16 = ml_dtypes.bfloat16
    xf = np.ascontiguousarray(np.asarray(x).reshape(TOK_TOTAL, HID))
    shared = {
        "wq": np.ascontiguousarray(np.asarray(Wq).T.reshape(HC, D, HID)).astype(bf16),
        "wkv": np.ascontiguousarray(
            np.concatenate([np.asarray(Wk).T, np.asarray(Wv).T], axis=1)
            .reshape(HC, D, 1024)
        ).astype(bf16),
        "wo": np.ascontiguousarray(np.asarray(Wo).T.reshape(HC, D, HID)).astype(bf16),
        "bqb": np.asarray(bq).reshape(1, HID).astype(bf16),
        "bkvb": np.concatenate([np.asarray(bk), np.asarray(bv)])
        .reshape(1, 1024).astype(bf16),
        "bob": np.asarray(bo).reshape(1, HID).astype(bf16),
        "ident": np.eye(D, dtype=np.float32).astype(bf16),
    }
    in_maps = []
    for c in range(N_CORES):
        xs = xf[c * TOK_CORE : (c + 1) * TOK_CORE]
        xt = np.ascontiguousarray(
            xs.reshape(N_ST, 128, HC, D).transpose(3, 0, 2, 1)
        ).astype(bf16)
        in_maps.append({"xt": xt, **shared})
    return in_maps


def kernel(x, Wq, bq, Wk, bk, Wv, bv, Wo, bo):
    x = np.asarray(x, dtype=np.float32)
    nc = _get_nc()
    in_maps = _prep_inputs(x, Wq, bq, Wk, bk, Wv, bv, Wo, bo)
    res = run_bass_kernel_spmd(nc, in_maps, core_ids=list(range(N_CORES)))
    y = np.concatenate([r["y"].astype(np.float32) for r in res.results], axis=0)
    return y.reshape(x.shape)
